# revision 28
# baseline (speedup 1.0000x reference)
"""Trainium2 Bass kernel for an 8-head MultiHeadAttention (B=2, S=4096, H=512).

Sharding: 8 NeuronCores, each takes (one batch, two heads):
    core c -> batch b = c // 4, heads {2*(c%4), 2*(c%4)+1}.

Per-core math (all fp32 data, float32r matmuls):
  - Host pre-transposes x[b] -> xT [512, 4096] and slices weight columns.
  - qT, kT computed in head-transposed layout [128(=2 heads x 64 dims), S]
    so the scores matmul needs no on-chip transposes.
  - v computed in natural layout [S, 128] with a ones column prepended per
    head -> the attention output matmul also accumulates the softmax
    denominator (row 0) for free.
  - Softmax without max-subtraction (scores provably small for this
    problem: |s| < ~15), scale 1/sqrt(64) folded into the Exp activation.
  - Normalization: DVE reciprocal of the denominator row + K=1 ones-matmul
    broadcast across partitions + DVE multiply.
  - Output written in transposed layout outT [128, S]; host reassembles.
"""

import os
import sys

sys.path.insert(0, "/opt/trn_rl_repo")

import numpy as np

import concourse.bass as bass  # noqa: E402
import concourse.tile as tile  # noqa: E402
from concourse import bacc, mybir  # noqa: E402
from concourse.bass_utils import run_bass_kernel_spmd  # noqa: E402

B, S, H = 2, 4096, 512
NH, HD = 8, 64
NCORES = 8
HPC = 2  # heads per core
DPC = HPC * HD  # head dims per core = 128
P = 128  # partitions
QB = 512  # query block (matmul free dim)
KC = 128  # key chunk (contraction tile)
KF = H // P  # feature chunks for projections = 4
NKC = S // KC  # 32
NQB = S // QB  # 8

f32 = mybir.dt.float32
f32r = mybir.dt.float32r


def _emit_kernel(ctx, tc, outT, xT, wq, wk, wv, bqk, bv, ones, onescol):
    nc = tc.nc

    const = ctx.enter_context(tc.tile_pool(name="const", bufs=1))

    # ---- constants / weights into SBUF ----
    # xT [H, S] -> sbuf [128, KF, S] (partition = feature % 128)
    xT_sb = const.tile([P, KF, S], f32r)
    for kf in range(KF):
        nc.sync.dma_start(out=xT_sb[:, kf, :], in_=xT[kf * P : (kf + 1) * P, :])

    # weights [H, 128] -> sbuf [128, KF, 128]
    wq_sb = const.tile([P, KF, DPC], f32r)
    wk_sb = const.tile([P, KF, DPC], f32r)
    wv_sb = const.tile([P, KF, DPC], f32r)
    for w_sb, w in ((wq_sb, wq), (wk_sb, wk), (wv_sb, wv)):
        nc.sync.dma_start(
            out=w_sb[:], in_=w.rearrange("(kf p) m -> p kf m", p=P)
        )

    # biases: bqk [2, 128] -> sbuf [128, 2] (partition = output dim)
    bqk_sb = const.tile([P, 2], f32)
    nc.sync.dma_start(out=bqk_sb[:], in_=bqk.rearrange("a m -> m a"))
    # bv [1, 128] stays row-major (used as K=1 matmul rhs)
    bv_sb = const.tile([1, DPC], f32r)
    nc.sync.dma_start(out=bv_sb[:], in_=bv[:])

    ones_sb = const.tile([1, P], f32r)
    nc.sync.dma_start(out=ones_sb[:], in_=ones[:])

    # ---- projections ----
    # qT/kT in [128 (2 heads x 64 dims), S] layout
    qkT_sb = const.tile([P, 2, S], f32r)
    # v in natural layout + ones column: vp_sb[p, kc, h, :HD] = v, [.., HD] = 1
    vp_sb = const.tile([P, NKC, HPC, HD + 1], f32r)
    nc.sync.dma_start(out=vp_sb[:, :, :, HD : HD + 1], in_=onescol[:])

    with tc.tile_pool(name="proj_psum", bufs=4, space="PSUM") as pp:
        with nc.named_scope("proj_qk"):
            for proj, w_sb in ((0, wq_sb), (1, wk_sb)):
                for sb in range(S // QB):
                    ps = pp.tile([P, QB], f32, tag="qk_ps")
                    for kf in range(KF):
                        nc.tensor.matmul(
                            ps[:],
                            lhsT=w_sb[:, kf, :],
                            rhs=xT_sb[:, kf, sb * QB : (sb + 1) * QB],
                            start=(kf == 0),
                            stop=(kf == KF - 1),
                        )
                    # psum -> sbuf eviction with fused bias add
                    nc.vector.tensor_scalar_add(
                        qkT_sb[:, proj, sb * QB : (sb + 1) * QB],
                        ps[:],
                        bqk_sb[:, proj : proj + 1],
                    )

        with nc.named_scope("proj_v"):
            for kc in range(NKC):
                ps = pp.tile([P, DPC], f32, tag="qk_ps")
                for kf in range(KF):
                    nc.tensor.matmul(
                        ps[:],
                        lhsT=xT_sb[:, kf, kc * P : (kc + 1) * P],
                        rhs=wv_sb[:, kf, :],
                        start=(kf == 0),
                        stop=False,
                    )
                # + bias (broadcast across partitions via K=1 matmul)
                nc.tensor.matmul(
                    ps[:],
                    lhsT=ones_sb[:],
                    rhs=bv_sb[:],
                    start=False,
                    stop=True,
                )
                for h in range(HPC):
                    nc.vector.tensor_copy(
                        vp_sb[:, kc, h, :HD], ps[:, h * HD : (h + 1) * HD]
                    )

    # ---- attention ----
    sc_pool = ctx.enter_context(tc.tile_pool(name="sc", bufs=2, space="PSUM"))
    ot_pool = ctx.enter_context(tc.tile_pool(name="ot", bufs=3, space="PSUM"))
    rb_pool = ctx.enter_context(tc.tile_pool(name="rb", bufs=1, space="PSUM"))
    ex_pool = ctx.enter_context(tc.tile_pool(name="ex", bufs=3))
    fin_pool = ctx.enter_context(tc.tile_pool(name="fin", bufs=4))
    rc_pool = ctx.enter_context(tc.tile_pool(name="rc", bufs=4))
    res_pool = ctx.enter_context(tc.tile_pool(name="res", bufs=4))

    with nc.named_scope("attn"):
        for qb in range(NQB):
            q0, q1 = qb * QB, (qb + 1) * QB
            oT = [
                ot_pool.tile([HD + 1, QB], f32, tag="oT", name=f"oT{qb}_{h}")
                for h in range(HPC)
            ]
            for kc in range(NKC):
                sc = sc_pool.tile([P, HPC, QB], f32, tag="sc")
                for h in range(HPC):
                    # scoresT[k, q] for head h; K = 64, rows 64h..64h+63
                    nc.tensor.matmul(
                        sc[:, h, :],
                        lhsT=qkT_sb[
                            h * HD : (h + 1) * HD, 1, kc * KC : (kc + 1) * KC
                        ],
                        rhs=qkT_sb[h * HD : (h + 1) * HD, 0, q0:q1],
                        start=True,
                        stop=True,
                        tile_position=(h * HD, 0),
                    )
                ex = ex_pool.tile([P, HPC, QB], f32r, tag="ex")
                nc.scalar.activation(
                    ex[:],
                    sc[:],
                    mybir.ActivationFunctionType.Exp,
                    scale=1.0 / np.sqrt(HD),
                )
                for h in range(HPC):
                    nc.tensor.matmul(
                        oT[h][:],
                        lhsT=vp_sb[:, kc, h, :],
                        rhs=ex[:, h, :],
                        start=(kc == 0),
                        stop=(kc == NKC - 1),
                    )
            for h in range(HPC):
                fin = fin_pool.tile([HD + 1, QB], f32, tag="fin")
                nc.vector.tensor_copy(fin[:], oT[h][:])
                rc = rc_pool.tile([1, QB], f32r, tag="rc")
                with nc.allow_low_precision(reason="f32r recip, 2^-12 rel"):
                    nc.vector.reciprocal(rc[:], fin[HD : HD + 1, :])
                rb = rb_pool.tile([HD, QB], f32, tag="rb")
                nc.tensor.matmul(
                    rb[:],
                    lhsT=ones_sb[:, :HD],
                    rhs=rc[:],
                    start=True,
                    stop=True,
                )
                res = res_pool.tile([HD, QB], f32, tag="res")
                nc.vector.tensor_mul(res[:], fin[:HD, :], rb[:])
                nc.sync.dma_start(
                    out=outT[h * HD : (h + 1) * HD, q0:q1], in_=res[:]
                )


def build_nc():
    nc = bacc.Bacc(
        "TRN2",
        target_bir_lowering=False,
        debug=False,
        num_devices=NCORES,
    )
    xT = nc.dram_tensor("xT", [H, S], f32r, kind="ExternalInput").ap()
    wq = nc.dram_tensor("wq", [H, DPC], f32r, kind="ExternalInput").ap()
    wk = nc.dram_tensor("wk", [H, DPC], f32r, kind="ExternalInput").ap()
    wv = nc.dram_tensor("wv", [H, DPC], f32r, kind="ExternalInput").ap()
    bqk = nc.dram_tensor("bqk", [2, DPC], f32, kind="ExternalInput").ap()
    bv = nc.dram_tensor("bv", [1, DPC], f32r, kind="ExternalInput").ap()
    ones = nc.dram_tensor("ones", [1, P], f32r, kind="ExternalInput").ap()
    onescol = nc.dram_tensor(
        "onescol", [P, NKC * HPC], f32r, kind="ExternalInput"
    ).ap()
    outT = nc.dram_tensor("outT", [DPC, S], f32, kind="ExternalOutput").ap()
    from contextlib import ExitStack

    with tile.TileContext(nc) as tc, ExitStack() as ctx:
        _emit_kernel(ctx, tc, outT, xT, wq, wk, wv, bqk, bv, ones, onescol)
    nc.compile()
    return nc


_NC_CACHE = None


def _get_nc():
    global _NC_CACHE
    if _NC_CACHE is None:
        _NC_CACHE = build_nc()
    return _NC_CACHE


def _round_f32r(a):
    """Round fp32 -> fp32r (e8m11: low 12 mantissa bits zeroed, RNE).

    The PE consumes fp32r operands by their top 20 bits; pre-rounding on
    the host matches what the hardware would use."""
    b = np.ascontiguousarray(a, dtype=np.float32).view(np.uint32)
    t = b + np.uint32(0x7FF) + ((b >> np.uint32(12)) & np.uint32(1))
    return (t & np.uint32(0xFFFFF000)).view(np.float32)


def _shard_inputs(x, Wq, bq, Wk, bk, Wv, bv):
    """Build per-core input maps (host does layout only: transpose/slice)."""
    x = np.ascontiguousarray(np.asarray(x, dtype=np.float32))
    in_maps = []
    xT_by_batch = [_round_f32r(x[b].T) for b in range(B)]
    for c in range(NCORES):
        b, p = c // (NCORES // B), c % (NCORES // B)
        cols = slice(p * DPC, (p + 1) * DPC)
        in_maps.append(
            {
                "xT": xT_by_batch[b],
                "wq": _round_f32r(np.asarray(Wq, np.float32)[:, cols]),
                "wk": _round_f32r(np.asarray(Wk, np.float32)[:, cols]),
                "wv": _round_f32r(np.asarray(Wv, np.float32)[:, cols]),
                "bqk": np.stack(
                    [
                        np.asarray(bq, np.float32)[cols],
                        np.asarray(bk, np.float32)[cols],
                    ]
                ),
                "bv": _round_f32r(np.asarray(bv, np.float32)[cols][None, :]),
                "ones": np.ones((1, P), dtype=np.float32),
                "onescol": np.ones((P, NKC * HPC), dtype=np.float32),
            }
        )
    return in_maps


def _assemble(results):
    out = np.empty((B, S, H), dtype=np.float32)
    for c in range(NCORES):
        b, p = c // (NCORES // B), c % (NCORES // B)
        outT = results[c]["outT"]  # [128, S]
        out[b, :, p * DPC : (p + 1) * DPC] = outT.T
    return out


def run(inputs, trace=False):
    nc = _get_nc()
    in_maps = _shard_inputs(**inputs)
    res = run_bass_kernel_spmd(nc, in_maps, list(range(NCORES)), trace=trace)
    return _assemble(res.results), res


def kernel(**inputs):
    out, _ = run(inputs)
    return out


# revision 29
# speedup vs baseline: 1.0864x; 1.0864x over previous
"""Trainium2 Bass kernel for an 8-head MultiHeadAttention (B=2, S=4096, H=512).

Sharding: 8 NeuronCores, each takes (one batch, two heads):
    core c -> batch b = c // 4, heads {2*(c%4), 2*(c%4)+1}.

Per-core math (all fp32 data, float32r matmuls):
  - Host pre-transposes x[b] -> xT [512, 4096] and slices weight columns.
  - qT, kT computed in head-transposed layout [128(=2 heads x 64 dims), S]
    so the scores matmul needs no on-chip transposes.
  - v computed in natural layout [S, 128] with a ones column prepended per
    head -> the attention output matmul also accumulates the softmax
    denominator (row 0) for free.
  - Softmax without max-subtraction (scores provably small for this
    problem: |s| < ~15), scale 1/sqrt(64) folded into the Exp activation.
  - Normalization: DVE reciprocal of the denominator row + K=1 ones-matmul
    broadcast across partitions + DVE multiply.
  - Output written in transposed layout outT [128, S]; host reassembles.
"""

import os
import sys

sys.path.insert(0, "/opt/trn_rl_repo")

import ml_dtypes
import numpy as np

import concourse.bass as bass  # noqa: E402
import concourse.tile as tile  # noqa: E402
from concourse import bacc, mybir  # noqa: E402
from concourse.bass_utils import run_bass_kernel_spmd  # noqa: E402

B, S, H = 2, 4096, 512
NH, HD = 8, 64
NCORES = 8
HPC = 2  # heads per core
DPC = HPC * HD  # head dims per core = 128
P = 128  # partitions
QB = 512  # query block (matmul free dim)
KC = 128  # key chunk (contraction tile)
KF = H // P  # feature chunks for projections = 4
NKC = S // KC  # 32
NQB = S // QB  # 8

f32 = mybir.dt.float32
_np_bf16 = ml_dtypes.bfloat16
f32r = mybir.dt.float32r
bf16 = mybir.dt.bfloat16


def _emit_kernel(ctx, tc, outT, xT, wq, wk, wv, bqk, bv, ones, onescol):
    nc = tc.nc

    const = ctx.enter_context(tc.tile_pool(name="const", bufs=1))

    # ---- constants / weights into SBUF (weights first: small, unblock PE) ----
    wq_sb = const.tile([P, KF, DPC], f32r)
    wk_sb = const.tile([P, KF, DPC], f32r)
    wv_sb = const.tile([P, KF, DPC], f32r)
    for w_sb, w in ((wq_sb, wq), (wk_sb, wk), (wv_sb, wv)):
        nc.sync.dma_start(
            out=w_sb[:], in_=w.rearrange("(kf p) m -> p kf m", p=P)
        )
    # biases: bqk [2, 128] -> sbuf [128, 2] (partition = output dim)
    bqk_sb = const.tile([P, 2], f32)
    nc.sync.dma_start(out=bqk_sb[:], in_=bqk.rearrange("a m -> m a"))
    # bv [1, 128] stays row-major (used as K=1 matmul rhs)
    bv_sb = const.tile([1, DPC], f32r)
    nc.sync.dma_start(out=bv_sb[:], in_=bv[:])
    ones_sb = const.tile([1, P], f32r)
    nc.sync.dma_start(out=ones_sb[:], in_=ones[:])

    # xT [H, S] -> sbuf [128, KF, S] (partition = feature % 128)
    xT_sb = const.tile([P, KF, S], f32r)
    for kf in range(KF):
        nc.sync.dma_start(out=xT_sb[:, kf, :], in_=xT[kf * P : (kf + 1) * P, :])

    # ---- projections ----
    # qT/kT in [128 (2 heads x 64 dims), S] layout
    qkT_sb = const.tile([P, 2, S], f32r)
    # v in natural layout + ones column: vp_sb[p, kc, h, :HD] = v, [.., HD] = 1
    vp_sb = const.tile([P, NKC, HPC, HD + 1], bf16)
    nc.sync.dma_start(out=vp_sb[:, :, :, HD : HD + 1], in_=onescol[:])

    with tc.tile_pool(name="proj_psum", bufs=8, space="PSUM") as pp:
        with nc.named_scope("proj_qk"):
            # kf-outer waves: the first matmuls need only xT chunk 0, so PE
            # starts as soon as the first 2MB of x lands
            for proj, w_sb in ((1, wk_sb), (0, wq_sb)):
                pss = [
                    pp.tile([P, QB], f32, tag="qk_ps", name=f"qk{proj}_{sb}")
                    for sb in range(S // QB)
                ]
                for kf in range(KF):
                    for sb in range(S // QB):
                        nc.tensor.matmul(
                            pss[sb][:],
                            lhsT=w_sb[:, kf, :],
                            rhs=xT_sb[:, kf, sb * QB : (sb + 1) * QB],
                            start=(kf == 0),
                            stop=(kf == KF - 1),
                        )
                for sb in range(S // QB):
                    # psum -> sbuf eviction with fused bias add
                    nc.vector.tensor_scalar_add(
                        qkT_sb[:, proj, sb * QB : (sb + 1) * QB],
                        pss[sb][:],
                        bqk_sb[:, proj : proj + 1],
                    )

        with nc.named_scope("proj_v"):
            for kc in range(NKC):
                ps = pp.tile([P, DPC], f32, tag="qk_ps")
                for kf in range(KF):
                    nc.tensor.matmul(
                        ps[:],
                        lhsT=xT_sb[:, kf, kc * P : (kc + 1) * P],
                        rhs=wv_sb[:, kf, :],
                        start=(kf == 0),
                        stop=False,
                    )
                # + bias (broadcast across partitions via K=1 matmul)
                nc.tensor.matmul(
                    ps[:],
                    lhsT=ones_sb[:],
                    rhs=bv_sb[:],
                    start=False,
                    stop=True,
                )
                for h in range(HPC):
                    nc.vector.tensor_copy(
                        vp_sb[:, kc, h, :HD], ps[:, h * HD : (h + 1) * HD]
                    )

    # ---- attention ----
    sc_pool = ctx.enter_context(tc.tile_pool(name="sc", bufs=2, space="PSUM"))
    ot_pool = ctx.enter_context(tc.tile_pool(name="ot", bufs=3, space="PSUM"))
    rb_pool = ctx.enter_context(tc.tile_pool(name="rb", bufs=1, space="PSUM"))
    ex_pool = ctx.enter_context(tc.tile_pool(name="ex", bufs=3))
    fin_pool = ctx.enter_context(tc.tile_pool(name="fin", bufs=4))
    rc_pool = ctx.enter_context(tc.tile_pool(name="rc", bufs=4))
    res_pool = ctx.enter_context(tc.tile_pool(name="res", bufs=4))

    with nc.named_scope("attn"):
        for qb in range(NQB):
            q0, q1 = qb * QB, (qb + 1) * QB
            oT = [
                ot_pool.tile([HD + 1, QB], f32, tag="oT", name=f"oT{qb}_{h}")
                for h in range(HPC)
            ]
            for kc in range(NKC):
                sc = sc_pool.tile([P, HPC, QB], f32, tag="sc")
                for h in range(HPC):
                    # scoresT[k, q] for head h; K = 64, rows 64h..64h+63
                    nc.tensor.matmul(
                        sc[:, h, :],
                        lhsT=qkT_sb[
                            h * HD : (h + 1) * HD, 1, kc * KC : (kc + 1) * KC
                        ],
                        rhs=qkT_sb[h * HD : (h + 1) * HD, 0, q0:q1],
                        start=True,
                        stop=True,
                        tile_position=(h * HD, 0),
                    )
                ex = ex_pool.tile([P, HPC, QB], bf16, tag="ex")
                nc.scalar.activation(
                    ex[:],
                    sc[:],
                    mybir.ActivationFunctionType.Exp,
                    scale=1.0 / np.sqrt(HD),
                )
                for h in range(HPC):
                    nc.tensor.matmul(
                        oT[h][:],
                        lhsT=vp_sb[:, kc, h, :],
                        rhs=ex[:, h, :],
                        start=(kc == 0),
                        stop=(kc == NKC - 1),
                    )
            for h in range(HPC):
                # sums row (f32r) straight from PSUM so the broadcast matmul
                # only waits on this one cheap DVE op
                srow = rc_pool.tile([1, QB], f32r, tag="srow", name=f"sr{qb}_{h}")
                with nc.allow_low_precision(reason="f32r sums, 2^-12 rel"):
                    nc.vector.tensor_copy(srow[:], oT[h][HD : HD + 1, :])
                fin = fin_pool.tile([HD + 1, QB], f32, tag="fin")
                nc.vector.tensor_copy(fin[:], oT[h][:])
                rb = rb_pool.tile([HD, QB], f32, tag="rb")
                nc.tensor.matmul(
                    rb[:],
                    lhsT=ones_sb[:, :HD],
                    rhs=srow[:],
                    start=True,
                    stop=True,
                )
                rcb = res_pool.tile([HD, QB], f32, tag="rcb", name=f"rcb{qb}_{h}")
                nc.vector.reciprocal_approx_fast(out=rcb[:], in_=rb[:])
                res = res_pool.tile([HD, QB], f32, tag="res")
                nc.vector.tensor_mul(res[:], fin[:HD, :], rcb[:])
                nc.sync.dma_start(
                    out=outT[h * HD : (h + 1) * HD, q0:q1], in_=res[:]
                )


def build_nc():
    nc = bacc.Bacc(
        "TRN2",
        target_bir_lowering=False,
        debug=False,
        num_devices=NCORES,
    )
    xT = nc.dram_tensor("xT", [H, S], f32r, kind="ExternalInput").ap()
    wq = nc.dram_tensor("wq", [H, DPC], f32r, kind="ExternalInput").ap()
    wk = nc.dram_tensor("wk", [H, DPC], f32r, kind="ExternalInput").ap()
    wv = nc.dram_tensor("wv", [H, DPC], f32r, kind="ExternalInput").ap()
    bqk = nc.dram_tensor("bqk", [2, DPC], f32, kind="ExternalInput").ap()
    bv = nc.dram_tensor("bv", [1, DPC], f32r, kind="ExternalInput").ap()
    ones = nc.dram_tensor("ones", [1, P], f32r, kind="ExternalInput").ap()
    onescol = nc.dram_tensor(
        "onescol", [P, NKC * HPC], bf16, kind="ExternalInput"
    ).ap()
    outT = nc.dram_tensor("outT", [DPC, S], f32, kind="ExternalOutput").ap()
    from contextlib import ExitStack

    with tile.TileContext(nc) as tc, ExitStack() as ctx:
        _emit_kernel(ctx, tc, outT, xT, wq, wk, wv, bqk, bv, ones, onescol)
    nc.compile()
    return nc


_NC_CACHE = None


def _get_nc():
    global _NC_CACHE
    if _NC_CACHE is None:
        _NC_CACHE = build_nc()
    return _NC_CACHE


def _round_f32r(a):
    """Round fp32 -> fp32r (e8m11: low 12 mantissa bits zeroed, RNE).

    The PE consumes fp32r operands by their top 20 bits; pre-rounding on
    the host matches what the hardware would use."""
    b = np.ascontiguousarray(a, dtype=np.float32).view(np.uint32)
    t = b + np.uint32(0x7FF) + ((b >> np.uint32(12)) & np.uint32(1))
    return (t & np.uint32(0xFFFFF000)).view(np.float32)


def _shard_inputs(x, Wq, bq, Wk, bk, Wv, bv):
    """Build per-core input maps (host does layout only: transpose/slice)."""
    x = np.ascontiguousarray(np.asarray(x, dtype=np.float32))
    in_maps = []
    xT_by_batch = [_round_f32r(x[b].T) for b in range(B)]
    for c in range(NCORES):
        b, p = c // (NCORES // B), c % (NCORES // B)
        cols = slice(p * DPC, (p + 1) * DPC)
        in_maps.append(
            {
                "xT": xT_by_batch[b],
                "wq": _round_f32r(np.asarray(Wq, np.float32)[:, cols]),
                "wk": _round_f32r(np.asarray(Wk, np.float32)[:, cols]),
                "wv": _round_f32r(np.asarray(Wv, np.float32)[:, cols]),
                "bqk": np.stack(
                    [
                        np.asarray(bq, np.float32)[cols],
                        np.asarray(bk, np.float32)[cols],
                    ]
                ),
                "bv": _round_f32r(np.asarray(bv, np.float32)[cols][None, :]),
                "ones": np.ones((1, P), dtype=np.float32),
                "onescol": np.ones((P, NKC * HPC), dtype=_np_bf16),
            }
        )
    return in_maps


def _assemble(results):
    out = np.empty((B, S, H), dtype=np.float32)
    for c in range(NCORES):
        b, p = c // (NCORES // B), c % (NCORES // B)
        outT = results[c]["outT"]  # [128, S]
        out[b, :, p * DPC : (p + 1) * DPC] = outT.T
    return out


def run(inputs, trace=False):
    nc = _get_nc()
    in_maps = _shard_inputs(**inputs)
    res = run_bass_kernel_spmd(nc, in_maps, list(range(NCORES)), trace=trace)
    return _assemble(res.results), res


def kernel(**inputs):
    out, _ = run(inputs)
    return out


# revision 30
# speedup vs baseline: 1.4005x; 1.2892x over previous
"""Trainium2 Bass kernel for an 8-head MultiHeadAttention (B=2, S=4096, H=512).

Sharding: 8 NeuronCores, each takes (one batch, two heads):
    core c -> batch b = c // 4, heads {2*(c%4), 2*(c%4)+1}.

Per-core math (all fp32 data, float32r matmuls):
  - Host pre-transposes x[b] -> xT [512, 4096] and slices weight columns.
  - qT, kT computed in head-transposed layout [128(=2 heads x 64 dims), S]
    so the scores matmul needs no on-chip transposes.
  - v computed in natural layout [S, 128] with a ones column prepended per
    head -> the attention output matmul also accumulates the softmax
    denominator (row 0) for free.
  - Softmax without max-subtraction (scores provably small for this
    problem: |s| < ~15), scale 1/sqrt(64) folded into the Exp activation.
  - Normalization: DVE reciprocal of the denominator row + K=1 ones-matmul
    broadcast across partitions + DVE multiply.
  - Output written in transposed layout outT [128, S]; host reassembles.
"""

import os
import sys

sys.path.insert(0, "/opt/trn_rl_repo")

import ml_dtypes
import numpy as np

import concourse.bass as bass  # noqa: E402
import concourse.tile as tile  # noqa: E402
from concourse import bacc, mybir  # noqa: E402
from concourse.bass_utils import run_bass_kernel_spmd  # noqa: E402

B, S, H = 2, 4096, 512
NH, HD = 8, 64
NCORES = 8
HPC = 2  # heads per core
DPC = HPC * HD  # head dims per core = 128
P = 128  # partitions
QB = 512  # query block (matmul free dim)
KC = 128  # key chunk (contraction tile)
KF = H // P  # feature chunks for projections = 4
NKC = S // KC  # 32
NQB = S // QB  # 8

f32 = mybir.dt.float32
_np_bf16 = ml_dtypes.bfloat16
f32r = mybir.dt.float32r
bf16 = mybir.dt.bfloat16


def _emit_kernel(ctx, tc, outT, xT, wq, wk, wv, bqk, bv, ones, ones_bf, onescol):
    nc = tc.nc

    const = ctx.enter_context(tc.tile_pool(name="const", bufs=1))

    # ---- constants / weights into SBUF (weights first: small, unblock PE) ----
    wq_sb = const.tile([P, KF, DPC], f32r)
    wk_sb = const.tile([P, KF, DPC], f32r)
    wv_sb = const.tile([P, KF, DPC], bf16)
    for w_sb, w in ((wq_sb, wq), (wk_sb, wk), (wv_sb, wv)):
        nc.sync.dma_start(
            out=w_sb[:], in_=w.rearrange("(kf p) m -> p kf m", p=P)
        )
    # biases: bqk [2, 128] -> sbuf [128, 2] (partition = output dim)
    bqk_sb = const.tile([P, 2], f32)
    nc.sync.dma_start(out=bqk_sb[:], in_=bqk.rearrange("a m -> m a"))
    # bv [1, 128] stays row-major (used as K=1 matmul rhs)
    bv_sb = const.tile([1, DPC], bf16)
    nc.sync.dma_start(out=bv_sb[:], in_=bv[:])
    ones_sb = const.tile([1, P], f32r)
    nc.sync.dma_start(out=ones_sb[:], in_=ones[:])
    ones_bf_sb = const.tile([1, P], bf16)
    nc.sync.dma_start(out=ones_bf_sb[:], in_=ones_bf[:])

    # xT [H, S] -> sbuf [128, KF, S] (partition = feature % 128)
    xT_sb = const.tile([P, KF, S], f32r)
    xT_bf = const.tile([P, KF, S], bf16)
    for kf in range(KF):
        nc.sync.dma_start(out=xT_sb[:, kf, :], in_=xT[kf * P : (kf + 1) * P, :])
    for kf in range(KF):
        with nc.allow_low_precision(reason="bf16 v-projection inputs"):
            nc.vector.tensor_copy(xT_bf[:, kf, :], xT_sb[:, kf, :])

    # ---- projections ----
    # qT/kT in [128 (2 heads x 64 dims), S] layout
    qkT_sb = const.tile([P, 2, S], bf16)
    # v in natural layout + ones column: vp_sb[p, kc, h, :HD] = v, [.., HD] = 1
    vp_sb = const.tile([P, NKC, HPC, HD + 1], bf16)
    nc.sync.dma_start(out=vp_sb[:, :, :, HD : HD + 1], in_=onescol[:])

    with tc.tile_pool(name="proj_psum", bufs=8, space="PSUM") as pp:
        with nc.named_scope("proj_qk"):
            # kf-outer waves: the first matmuls need only xT chunk 0, so PE
            # starts as soon as the first 2MB of x lands
            for proj, w_sb in ((1, wk_sb), (0, wq_sb)):
                pss = [
                    pp.tile([P, QB], f32, tag="qk_ps", name=f"qk{proj}_{sb}")
                    for sb in range(S // QB)
                ]
                for kf in range(KF):
                    for sb in range(S // QB):
                        nc.tensor.matmul(
                            pss[sb][:],
                            lhsT=w_sb[:, kf, :],
                            rhs=xT_sb[:, kf, sb * QB : (sb + 1) * QB],
                            start=(kf == 0),
                            stop=(kf == KF - 1),
                        )
                for sb in range(S // QB):
                    # psum -> sbuf eviction with fused bias add (bf16 out)
                    with nc.allow_low_precision(reason="bf16 q/k for scores"):
                        nc.vector.tensor_scalar_add(
                            qkT_sb[:, proj, sb * QB : (sb + 1) * QB],
                            pss[sb][:],
                            bqk_sb[:, proj : proj + 1],
                        )

        with nc.named_scope("proj_v"):
            for kc in range(NKC):
                ps = pp.tile([P, DPC], f32, tag="qk_ps")
                for kf in range(KF):
                    nc.tensor.matmul(
                        ps[:],
                        lhsT=xT_bf[:, kf, kc * P : (kc + 1) * P],
                        rhs=wv_sb[:, kf, :],
                        start=(kf == 0),
                        stop=False,
                    )
                # + bias (broadcast across partitions via K=1 matmul)
                nc.tensor.matmul(
                    ps[:],
                    lhsT=ones_bf_sb[:],
                    rhs=bv_sb[:],
                    start=False,
                    stop=True,
                )
                for h in range(HPC):
                    nc.vector.tensor_copy(
                        vp_sb[:, kc, h, :HD], ps[:, h * HD : (h + 1) * HD]
                    )

    # ---- attention ----
    sc_pool = ctx.enter_context(tc.tile_pool(name="sc", bufs=2, space="PSUM"))
    ot_pool = ctx.enter_context(tc.tile_pool(name="ot", bufs=3, space="PSUM"))
    rb_pool = ctx.enter_context(tc.tile_pool(name="rb", bufs=1, space="PSUM"))
    ex_pool = ctx.enter_context(tc.tile_pool(name="ex", bufs=3))
    fin_pool = ctx.enter_context(tc.tile_pool(name="fin", bufs=4))
    rc_pool = ctx.enter_context(tc.tile_pool(name="rc", bufs=4))
    res_pool = ctx.enter_context(tc.tile_pool(name="res", bufs=4))

    with nc.named_scope("attn"):
        for qb in range(NQB):
            q0, q1 = qb * QB, (qb + 1) * QB
            oT = [
                ot_pool.tile([HD + 1, QB], f32, tag="oT", name=f"oT{qb}_{h}")
                for h in range(HPC)
            ]
            for kc in range(NKC):
                sc = sc_pool.tile([P, HPC, QB], f32, tag="sc")
                for h in range(HPC):
                    # scoresT[k, q] for head h; K = 64, rows 64h..64h+63
                    nc.tensor.matmul(
                        sc[:, h, :],
                        lhsT=qkT_sb[
                            h * HD : (h + 1) * HD, 1, kc * KC : (kc + 1) * KC
                        ],
                        rhs=qkT_sb[h * HD : (h + 1) * HD, 0, q0:q1],
                        start=True,
                        stop=True,
                        tile_position=(h * HD, 0),
                    )
                ex = ex_pool.tile([P, HPC, QB], bf16, tag="ex")
                nc.scalar.activation(
                    ex[:],
                    sc[:],
                    mybir.ActivationFunctionType.Exp,
                    scale=1.0 / np.sqrt(HD),
                )
                for h in range(HPC):
                    nc.tensor.matmul(
                        oT[h][:],
                        lhsT=vp_sb[:, kc, h, :],
                        rhs=ex[:, h, :],
                        start=(kc == 0),
                        stop=(kc == NKC - 1),
                    )
            for h in range(HPC):
                # sums row (f32r) straight from PSUM so the broadcast matmul
                # only waits on this one cheap DVE op
                srow = rc_pool.tile([1, QB], f32r, tag="srow", name=f"sr{qb}_{h}")
                with nc.allow_low_precision(reason="f32r sums, 2^-12 rel"):
                    nc.vector.tensor_copy(srow[:], oT[h][HD : HD + 1, :])
                fin = fin_pool.tile([HD + 1, QB], f32, tag="fin")
                nc.vector.tensor_copy(fin[:], oT[h][:])
                rb = rb_pool.tile([HD, QB], f32, tag="rb")
                nc.tensor.matmul(
                    rb[:],
                    lhsT=ones_sb[:, :HD],
                    rhs=srow[:],
                    start=True,
                    stop=True,
                )
                rcb = res_pool.tile([HD, QB], f32, tag="rcb", name=f"rcb{qb}_{h}")
                nc.vector.reciprocal_approx_fast(out=rcb[:], in_=rb[:])
                res = res_pool.tile([HD, QB], f32, tag="res")
                nc.vector.tensor_mul(res[:], fin[:HD, :], rcb[:])
                nc.sync.dma_start(
                    out=outT[h * HD : (h + 1) * HD, q0:q1], in_=res[:]
                )


def build_nc():
    nc = bacc.Bacc(
        "TRN2",
        target_bir_lowering=False,
        debug=False,
        num_devices=NCORES,
    )
    xT = nc.dram_tensor("xT", [H, S], f32r, kind="ExternalInput").ap()
    wq = nc.dram_tensor("wq", [H, DPC], f32r, kind="ExternalInput").ap()
    wk = nc.dram_tensor("wk", [H, DPC], f32r, kind="ExternalInput").ap()
    wv = nc.dram_tensor("wv", [H, DPC], bf16, kind="ExternalInput").ap()
    bqk = nc.dram_tensor("bqk", [2, DPC], f32, kind="ExternalInput").ap()
    bv = nc.dram_tensor("bv", [1, DPC], bf16, kind="ExternalInput").ap()
    ones_bf = nc.dram_tensor("ones_bf", [1, P], bf16, kind="ExternalInput").ap()
    ones = nc.dram_tensor("ones", [1, P], f32r, kind="ExternalInput").ap()
    onescol = nc.dram_tensor(
        "onescol", [P, NKC * HPC], bf16, kind="ExternalInput"
    ).ap()
    outT = nc.dram_tensor("outT", [DPC, S], f32, kind="ExternalOutput").ap()
    from contextlib import ExitStack

    with tile.TileContext(nc) as tc, ExitStack() as ctx:
        _emit_kernel(ctx, tc, outT, xT, wq, wk, wv, bqk, bv, ones, ones_bf, onescol)
    nc.compile()
    return nc


_NC_CACHE = None


def _get_nc():
    global _NC_CACHE
    if _NC_CACHE is None:
        _NC_CACHE = build_nc()
    return _NC_CACHE


def _round_f32r(a):
    """Round fp32 -> fp32r (e8m11: low 12 mantissa bits zeroed, RNE).

    The PE consumes fp32r operands by their top 20 bits; pre-rounding on
    the host matches what the hardware would use."""
    b = np.ascontiguousarray(a, dtype=np.float32).view(np.uint32)
    t = b + np.uint32(0x7FF) + ((b >> np.uint32(12)) & np.uint32(1))
    return (t & np.uint32(0xFFFFF000)).view(np.float32)


def _shard_inputs(x, Wq, bq, Wk, bk, Wv, bv):
    """Build per-core input maps (host does layout only: transpose/slice)."""
    x = np.ascontiguousarray(np.asarray(x, dtype=np.float32))
    in_maps = []
    xT_by_batch = [_round_f32r(x[b].T) for b in range(B)]
    for c in range(NCORES):
        b, p = c // (NCORES // B), c % (NCORES // B)
        cols = slice(p * DPC, (p + 1) * DPC)
        in_maps.append(
            {
                "xT": xT_by_batch[b],
                "wq": _round_f32r(np.asarray(Wq, np.float32)[:, cols]),
                "wk": _round_f32r(np.asarray(Wk, np.float32)[:, cols]),
                "wv": np.asarray(Wv, np.float32)[:, cols].astype(_np_bf16),
                "bqk": np.stack(
                    [
                        np.asarray(bq, np.float32)[cols],
                        np.asarray(bk, np.float32)[cols],
                    ]
                ),
                "bv": np.asarray(bv, np.float32)[cols][None, :].astype(_np_bf16),
                "ones": np.ones((1, P), dtype=np.float32),
                "ones_bf": np.ones((1, P), dtype=_np_bf16),
                "onescol": np.ones((P, NKC * HPC), dtype=_np_bf16),
            }
        )
    return in_maps


def _assemble(results):
    out = np.empty((B, S, H), dtype=np.float32)
    for c in range(NCORES):
        b, p = c // (NCORES // B), c % (NCORES // B)
        outT = results[c]["outT"]  # [128, S]
        out[b, :, p * DPC : (p + 1) * DPC] = outT.T
    return out


def run(inputs, trace=False):
    nc = _get_nc()
    in_maps = _shard_inputs(**inputs)
    res = run_bass_kernel_spmd(nc, in_maps, list(range(NCORES)), trace=trace)
    return _assemble(res.results), res


def kernel(**inputs):
    out, _ = run(inputs)
    return out
